# revision 30
# baseline (speedup 1.0000x reference)
"""Bass/Trainium2 kernel for a 2-layer GCN encoder (PyG GCNConv semantics).

Strategy (graph/data parallel over 8 NeuronCores):
  - Nodes are range-sharded: core c owns dst nodes [c*12500, (c+1)*12500).
  - With the dinv-prescaled features x~ = dinv[:,None]*x and table
    zt2 = dinv[:,None]*(h1 @ W2), each layer is
        h1_i  = relu(dinv_i*((sum_{e->i} x~[src_e] + x~_i) @ W1) + b1)
        out_i = dinv_i*( sum_{e->i} zt2[src_e] + zt2_i ) + b2
    (aggregate-then-transform via linearity for layer 1).
  - Layer 1 messages are HOST-MARSHALLED: x~[src] rows are shipped in
    edge-slot order (halo exchange materialized on the host), so the
    device consumes them with big affine DMAs and scatter-accumulates
    via one-hot matmuls (mask[e,j] = (dst_local[e]==j) built on VectorE
    from a host dst_local stream vs an iota constant, PSUM accumulates).
    Layer-1 slots are packed per dst block (no quarter structure).
  - Layer 2 messages are device-gathered (h1 is device-resident): each
    core computes zt2 for its shard, an AllGather replicates the table
    (fp16), and dma_gather fetches edge-source rows. int16 gather
    indices limit a call to 32767 rows, so the padded 100352-row table
    is split in 4 quarters of 25088 rows. One call per (quarter,
    superblock of SB_C=4 blocks): the four blocks' real edges are
    packed block-major and pads (idx 0, dst_local -1) trail. The Q7
    generation cost is ~8.3ns per slot regardless of validity, so
    packing across 4 blocks minimizes slots. Per chunk ONE wide
    [128, 512] mask op (dst_local value j + 128*bi vs an iota512
    constant) feeds four matmuls, one per block PSUM.
  - All cores run one SPMD NEFF: slot counts are padded to the max over
    cores so the program is identical everywhere.
"""

import sys

import numpy as np

sys.path.insert(0, "/opt/trn_rl_repo")

N_NODES = 100000
N_EDGES = 1600000
D_IN, D_HID, D_OUT = 256, 256, 128
N_CORES = 8
NC_NODES = N_NODES // N_CORES  # 12500 real nodes per core
NP = 12544  # padded nodes per core (98 blocks of 128)
NBLK = NP // 128  # 98
NROWS = N_CORES * NP  # 100352 padded table rows
NQ = 4
QS = NROWS // NQ  # 25088 rows per quarter (< 32767 for int16 idx)
SB_B = 2  # dst blocks per superblock, layer-1 stream batching
SB_C = 4  # dst blocks per superblock, layer-2 gather calls


def _pad_row(n):
    return (n // NC_NODES) * NP + (n % NC_NODES)


def _sbs(sb):
    return [list(range(s, min(s + sb, NBLK))) for s in range(0, NBLK, sb)]


def build_layout_b(counts_b):
    """counts_b: [N_CORES, NBLK] edges per dst block. Layer-1 layout:
    slots packed per block (quarters irrelevant), padded to x128 at the
    max over cores."""
    maxc = counts_b.max(axis=0)
    nch_blk = np.ceil(maxc / 128.0).astype(np.int64)  # chunks per block
    ch_off = np.zeros(NBLK + 1, dtype=np.int64)
    np.cumsum(nch_blk, out=ch_off[1:])
    layout = []
    for blocks in _sbs(SB_B):
        layout.append(
            dict(
                blocks=blocks,
                ch0=int(ch_off[blocks[0]]),
                nch=int(sum(nch_blk[b] for b in blocks)),
                blk_chunks={b: (int(ch_off[b]), int(nch_blk[b])) for b in blocks},
            )
        )
    return dict(
        layout=layout,
        nch=int(ch_off[-1]),
        nslots=int(ch_off[-1]) * 128,
        slot_off=ch_off[:-1] * 128,  # per block
    )


def build_layout_c(cnt_c4):
    """cnt_c4: [N_CORES, NSB, NQ, SB_C] edges per (sb, quarter, block).
    Layer-2 layout: per (sb, q) one gather call; real slots packed
    block-major, pads (idx 0) trail; padded to x128 at the max over
    cores. Each chunk records the union (over cores) of blocks whose
    slot range intersects it, so the device only emits mask+matmul for
    (chunk, block) pairs that can be non-zero on some core."""
    counts_c = cnt_c4.sum(axis=3).transpose(0, 2, 1)  # [cores, NQ, NSB]
    maxc = counts_c.max(axis=0)  # [NQ, NSB]
    s_call = 128 * np.ceil(maxc / 128.0).astype(np.int64)
    layout = []
    gch = 0
    call_id = 0
    sbs = _sbs(SB_C)
    slot_off = np.zeros((NQ, len(sbs)), dtype=np.int64)
    for si, blocks in enumerate(sbs):
        sb_ch0 = gch
        calls = []
        for q in range(NQ):
            s = int(s_call[q][si])
            if s == 0:
                continue
            slot_off[q][si] = gch * 128
            # per-core block boundaries within this call (slot space)
            bounds = np.zeros((N_CORES, len(blocks) + 1), dtype=np.int64)
            np.cumsum(cnt_c4[:, si, q, : len(blocks)], axis=1, out=bounds[:, 1:])
            chunk_blocks = []
            for t in range(s // 128):
                lo, hi = 128 * t, 128 * (t + 1)
                touched = set()
                for c in range(N_CORES):
                    for bi in range(len(blocks)):
                        if bounds[c][bi] < hi and bounds[c][bi + 1] > lo:
                            touched.add(bi)
                chunk_blocks.append(sorted(touched))
            calls.append(
                dict(
                    q=q,
                    ioff16=gch * 8,
                    s=s,
                    mcol=gch - sb_ch0,
                    call_id=call_id,
                    chunk_blocks=chunk_blocks,
                )
            )
            call_id += 1
            gch += s // 128
        layout.append(
            dict(blocks=blocks, ch0=sb_ch0, nch=gch - sb_ch0, calls=calls)
        )
    return dict(
        layout=layout,
        nch=gch,
        nslots=gch * 128,
        ncalls=call_id,
        slot_off=slot_off,
    )


def preprocess(x, edge_index, W1, b1, W2, b2):
    """Host-side sharding/marshalling. Returns (in_maps, (lb, lc))."""
    src = np.asarray(edge_index[0], dtype=np.int64)
    dst = np.asarray(edge_index[1], dtype=np.int64)
    x = np.asarray(x)
    W1 = np.asarray(W1)
    b1 = np.asarray(b1)
    W2 = np.asarray(W2)
    b2 = np.asarray(b2)

    deg = np.bincount(dst, minlength=N_NODES).astype(np.float32) + 1.0
    dinv = (1.0 / np.sqrt(deg)).astype(np.float32)
    xt = (x.astype(np.float32) * dinv[:, None]).astype(np.float16)

    core = dst // NC_NODES
    dstl = dst % NC_NODES
    blk = dstl // 128
    j = (dstl % 128).astype(np.int64)
    prow = _pad_row(src)
    q = prow // QS
    sq = (prow % QS).astype(np.int64)
    sb_of_blk = blk // SB_C
    bi = blk % SB_C
    nsb = (NBLK + SB_C - 1) // SB_C

    # ----- layer-1 (B) layout: group by (core, blk) -----
    key_b = core * NBLK + blk
    order_b = np.argsort(key_b, kind="stable")
    cnt_b = np.bincount(key_b, minlength=N_CORES * NBLK).reshape(N_CORES, NBLK)
    lb = build_layout_b(cnt_b)
    gstart = np.zeros(N_CORES * NBLK + 1, dtype=np.int64)
    np.cumsum(cnt_b.reshape(-1), out=gstart[1:])
    rank_b = np.arange(len(src)) - gstart[key_b[order_b]]
    pos_b = lb["slot_off"][blk[order_b]] + rank_b  # slot in core's B stream

    # ----- layer-2 (C) layout: group by (core, sb, q, bi) -----
    key_c = ((core * nsb + sb_of_blk) * NQ + q) * SB_C + bi
    order_c = np.argsort(key_c, kind="stable")
    ngroups_c = N_CORES * nsb * NQ * SB_C
    cnt_c4 = np.bincount(key_c, minlength=ngroups_c).reshape(
        N_CORES, nsb, NQ, SB_C
    )
    lc = build_layout_c(cnt_c4)
    # rank within (core, sb, q) with blocks in bi order: cumulative offsets
    gstart_c = np.zeros(ngroups_c + 1, dtype=np.int64)
    np.cumsum(cnt_c4.reshape(-1), out=gstart_c[1:])
    key_cs = key_c[order_c]
    grp_base = (key_cs // SB_C) * SB_C  # index of bi=0 group
    off_in_call = gstart_c[key_cs] - gstart_c[grp_base]
    rank_c = np.arange(len(src)) - gstart_c[key_cs]
    pos_c_all = (
        lc["slot_off"][q[order_c], sb_of_blk[order_c]] + off_in_call + rank_c
    )

    in_maps = []
    iota_np = np.tile(np.arange(128, dtype=np.float16)[None, :], (128, 1))
    iotaw_np = np.tile(
        np.arange(128 * SB_C, dtype=np.float16)[None, :], (128, 1)
    )
    ident_np = np.eye(128, dtype=np.float16)
    W1h = W1.astype(np.float16).reshape(D_IN // 128, 128, D_HID).transpose(1, 0, 2).copy()
    W2h = W2.astype(np.float16).reshape(D_HID // 128, 128, D_OUT).transpose(1, 0, 2).copy()
    b1b = np.tile(b1.astype(np.float32)[None, :], (128, 1))
    b2b = np.tile(b2.astype(np.float32)[None, :], (128, 1))

    core_bs = core[order_b]
    core_cs = core[order_c]

    for c in range(N_CORES):
        # --- B stream: marshalled x~[src] rows + dst_local values ---
        m_b = core_bs == c
        posb = pos_b[m_b]
        nslB, nchB = lb["nslots"], lb["nch"]
        dlb = np.full(nslB, -1.0, dtype=np.float32)
        dlb[posb] = j[order_b][m_b].astype(np.float32)
        dlb_w = dlb.reshape(-1, 128).T.copy()
        xs = np.zeros((nslB, D_IN), dtype=np.float16)
        xs[posb] = xt[src[order_b][m_b]]
        xs = np.ascontiguousarray(xs.reshape(nchB, 128, D_IN).transpose(1, 0, 2))

        # --- C stream: gather idx + dual-block dst_local + counts ---
        m_c = core_cs == c
        posc = pos_c_all[m_c]
        nslC, nchC = lc["nslots"], lc["nch"]
        idxc = np.zeros(nslC, dtype=np.int16)
        idxc[posc] = sq[order_c][m_c].astype(np.int16)
        dlc = np.full(nslC, -1.0, dtype=np.float32)
        dlc[posc] = (j[order_c][m_c] + 128 * bi[order_c][m_c]).astype(
            np.float32
        )
        dlc_w = dlc.reshape(-1, 128).T.copy()
        idxc_w = np.tile(idxc.reshape(-1, 16).T, (8, 1)).copy()

        # --- own-shard x~ rows for the L1 self term ---
        xself = np.zeros((NP, D_IN), dtype=np.float16)
        xself[:NC_NODES] = xt[c * NC_NODES : (c + 1) * NC_NODES]
        xself = np.ascontiguousarray(
            xself.reshape(NBLK, 128, D_IN).transpose(1, 0, 2)
        )

        dinv_c = np.zeros((128, NBLK), dtype=np.float32)
        dv = np.zeros(NP, dtype=np.float32)
        dv[:NC_NODES] = dinv[c * NC_NODES : (c + 1) * NC_NODES]
        dinv_c[:, :] = dv.reshape(NBLK, 128).T

        in_maps.append(
            dict(
                xs=xs,
                xself=xself,
                W1h=W1h,
                W2h=W2h,
                b1b=b1b,
                b2b=b2b,
                iota=iota_np,
                iotaw=iotaw_np,
                ident=ident_np,
                dinv=dinv_c,
                eidx=idxc_w,
                edlB=dlb_w,
                edlC=dlc_w,
            )
        )
    return in_maps, (lb, lc)


def build_nc(layout_info):
    import concourse.tile as tile
    from concourse import bacc, mybir

    lb, lc = layout_info
    nchB = lb["nch"]
    nchC, nslotsC = lc["nch"], lc["nslots"]
    f16 = mybir.dt.float16
    f32 = mybir.dt.float32
    i16 = mybir.dt.int16

    nc = bacc.Bacc(
        "TRN2", target_bir_lowering=False, debug=False, num_devices=N_CORES
    )
    xs = nc.dram_tensor("xs", [128, nchB, D_IN], f16, kind="ExternalInput").ap()
    xself = nc.dram_tensor("xself", [128, NBLK, D_IN], f16, kind="ExternalInput").ap()
    W1h = nc.dram_tensor("W1h", [128, D_IN // 128, D_HID], f16, kind="ExternalInput").ap()
    W2h = nc.dram_tensor("W2h", [128, D_HID // 128, D_OUT], f16, kind="ExternalInput").ap()
    b1b = nc.dram_tensor("b1b", [128, D_HID], f32, kind="ExternalInput").ap()
    b2b = nc.dram_tensor("b2b", [128, D_OUT], f32, kind="ExternalInput").ap()
    iota = nc.dram_tensor("iota", [128, 128], f16, kind="ExternalInput").ap()
    iotaw = nc.dram_tensor("iotaw", [128, 128 * SB_C], f16, kind="ExternalInput").ap()
    ident = nc.dram_tensor("ident", [128, 128], f16, kind="ExternalInput").ap()
    dinv = nc.dram_tensor("dinv", [128, NBLK], f32, kind="ExternalInput").ap()
    eidx = nc.dram_tensor("eidx", [128, nslotsC // 16], i16, kind="ExternalInput").ap()
    edlB = nc.dram_tensor("edlB", [128, nchB], f32, kind="ExternalInput").ap()
    edlC = nc.dram_tensor("edlC", [128, nchC], f32, kind="ExternalInput").ap()
    out = nc.dram_tensor("out", [NP, D_OUT], f32, kind="ExternalOutput").ap()

    zt2_c = nc.dram_tensor("zt2_c", [NP, D_OUT], f16)
    zt2_full = nc.dram_tensor("zt2_full", [NROWS, D_OUT], f16, addr_space="Shared")

    with tile.TileContext(nc) as tc:
        consts = tc.alloc_tile_pool(name="consts", bufs=1)
        w1_t = consts.tile([128, D_IN // 128, D_HID], f16)
        nc.sync.dma_start(w1_t[:], W1h[:, :, :])
        w2_t = consts.tile([128, D_HID // 128, D_OUT], f16)
        nc.sync.dma_start(w2_t[:], W2h[:, :, :])
        b1_t = consts.tile([128, D_HID], f32)
        nc.sync.dma_start(b1_t[:], b1b[:, :])
        b2_t = consts.tile([128, D_OUT], f32)
        nc.sync.dma_start(b2_t[:], b2b[:, :])
        iota_t = consts.tile([128, 128], f16)
        nc.sync.dma_start(iota_t[:], iota[:, :])
        iotaw_t = consts.tile([128, 128 * SB_C], f16)
        nc.sync.dma_start(iotaw_t[:], iotaw[:, :])
        ident_t = consts.tile([128, 128], f16)
        nc.sync.dma_start(ident_t[:], ident[:, :])
        dinv_t = consts.tile([128, NBLK], f32)
        nc.sync.dma_start(dinv_t[:], dinv[:, :])
        dlb_t = consts.tile([128, nchB], f32)
        nc.sync.dma_start(dlb_t[:], edlB[:, :])

        def make_mask(maskp, dl_t, gc, base_t):
            mask = maskp.tile([128, 128], f16, tag="mask")
            nc.vector.tensor_scalar(
                out=mask[:], in0=base_t[:], scalar1=dl_t[:, gc : gc + 1],
                scalar2=None, op0=mybir.AluOpType.is_equal,
            )
            return mask

        # ------------- Phase B: L1 aggregate-then-transform + zt2 -------------
        with tc.tile_pool(name="msgB", bufs=4) as msgp, \
             tc.tile_pool(name="maskB", bufs=10) as maskp, \
             tc.tile_pool(name="selfB", bufs=4) as selfp, \
             tc.tile_pool(name="psumX", bufs=2, space="PSUM") as psumX, \
             tc.tile_pool(name="psumT", bufs=2, space="PSUM") as psumT, \
             tc.tile_pool(name="psumZ", bufs=2, space="PSUM") as psumZ, \
             tc.tile_pool(name="epiB", bufs=6) as epi:
            for sbl in lb["layout"]:
                msg = msgp.tile([128, sbl["nch"], D_IN], f16, tag="msg")
                nc.sync.dma_start(
                    msg[:], xs[:, sbl["ch0"] : sbl["ch0"] + sbl["nch"], :]
                )
                for b in sbl["blocks"]:
                    ch0, nch_b = sbl["blk_chunks"][b]
                    lc0 = ch0 - sbl["ch0"]
                    self_t = selfp.tile([128, D_IN], f16, tag="self")
                    nc.sync.dma_start(self_t[:], xself[:, b, :])
                    psx = psumX.tile([128, D_IN], f32, tag="aggx")
                    for t in range(nch_b):
                        if t % 3 != 2:
                            mask = make_mask(maskp, dlb_t, ch0 + t, iota_t)
                        else:
                            dl_col = dlb_t[:, ch0 + t : ch0 + t + 1]
                            adiff = maskp.tile([128, 128], f16, tag="adiffB")
                            nc.scalar.activation(
                                adiff[:], iota_t[:],
                                mybir.ActivationFunctionType.Abs,
                                bias=dl_col, scale=-1.0,
                            )
                            mask = maskp.tile([128, 128], f16, tag="mask")
                            nc.scalar.activation(
                                mask[:], adiff[:],
                                mybir.ActivationFunctionType.Relu,
                                bias=1.0, scale=-1.0,
                            )
                        nc.tensor.matmul(
                            psx[:], lhsT=mask[:], rhs=msg[:, lc0 + t, :],
                            start=(t == 0), stop=False,
                        )
                    nc.tensor.matmul(
                        psx[:], lhsT=ident_t[:], rhs=self_t[:],
                        start=(nch_b == 0), stop=True,
                    )
                    # aggx (psum f32) -> fp16 sbuf -> transpose -> @W1
                    aggx = epi.tile([128, D_IN], f16, tag="aggx16")
                    nc.scalar.activation(
                        aggx[:], psx[:], mybir.ActivationFunctionType.Copy
                    )
                    aggxT = epi.tile([128, D_IN // 128, 128], f16, tag="aggxT")
                    for k in range(D_IN // 128):
                        pst = psumT.tile([128, 128], f16, tag="pst")
                        nc.tensor.transpose(
                            pst[:], aggx[:, k * 128 : (k + 1) * 128], ident_t[:]
                        )
                        nc.vector.tensor_copy(aggxT[:, k, :], pst[:])
                    psz = psumZ.tile([128, D_HID], f32, tag="psz")
                    for k in range(D_IN // 128):
                        nc.tensor.matmul(
                            psz[:], lhsT=aggxT[:, k, :], rhs=w1_t[:, k, :],
                            start=(k == 0), stop=(k == D_IN // 128 - 1),
                        )
                    # h1 = relu(dinv * psz + b1)
                    t1 = epi.tile([128, D_HID], f32, tag="t1")
                    nc.vector.tensor_scalar(
                        out=t1[:], in0=psz[:], scalar1=dinv_t[:, b : b + 1],
                        scalar2=None, op0=mybir.AluOpType.mult,
                    )
                    nc.vector.tensor_tensor(
                        out=t1[:], in0=t1[:], in1=b1_t[:], op=mybir.AluOpType.add
                    )
                    h1 = epi.tile([128, D_HID], f16, tag="h1")
                    nc.scalar.activation(
                        h1[:], t1[:], mybir.ActivationFunctionType.Relu
                    )
                    # zt2 = dinv * (h1 @ W2)
                    h1T = epi.tile([128, D_HID // 128, 128], f16, tag="h1T")
                    for k in range(D_HID // 128):
                        pst = psumT.tile([128, 128], f16, tag="pst")
                        nc.tensor.transpose(
                            pst[:], h1[:, k * 128 : (k + 1) * 128], ident_t[:]
                        )
                        nc.vector.tensor_copy(h1T[:, k, :], pst[:])
                    ps2 = psumZ.tile([128, D_OUT], f32, tag="ps2")
                    for k in range(D_HID // 128):
                        nc.tensor.matmul(
                            ps2[:], lhsT=h1T[:, k, :], rhs=w2_t[:, k, :],
                            start=(k == 0), stop=(k == D_HID // 128 - 1),
                        )
                    zt2 = epi.tile([128, D_OUT], f16, tag="zt2")
                    nc.vector.tensor_scalar(
                        out=zt2[:], in0=ps2[:], scalar1=dinv_t[:, b : b + 1],
                        scalar2=None, op0=mybir.AluOpType.mult,
                    )
                    nc.sync.dma_start(
                        zt2_c.ap()[b * 128 : (b + 1) * 128, :], zt2[:]
                    )

        tc.strict_bb_all_engine_barrier()
        with tc.tile_critical():
            with nc.semaphore("cc2") as cc2:
                nc.gpsimd.collective_compute(
                    "AllGather",
                    mybir.AluOpType.bypass,
                    replica_groups=[list(range(N_CORES))],
                    ins=[zt2_c.ap().opt()],
                    outs=[zt2_full.ap().opt()],
                ).then_inc(cc2)
                nc.gpsimd.wait_ge(cc2, 1)
        tc.strict_bb_all_engine_barrier()

        # ---------------- Phase C: L2 aggregation -> out ----------------
        idx_t = consts.tile([128, nslotsC // 16], i16)
        nc.sync.dma_start(idx_t[:], eidx[:, :])
        dlc_t = consts.tile([128, nchC], f32)
        nc.sync.dma_start(dlc_t[:], edlC[:, :])

        with tc.tile_pool(name="msgC", bufs=4) as msgp, \
             tc.tile_pool(name="maskC", bufs=10) as maskp, \
             tc.tile_pool(name="selfC", bufs=4) as selfp, \
             tc.tile_pool(name="psumC", bufs=2 * SB_C, space="PSUM") as psumC, \
             tc.tile_pool(name="epiC", bufs=4) as epi:
            for sbi, sbl in enumerate(lc["layout"]):
                msg = msgp.tile([128, sbl["nch"], D_OUT], f16, tag="msg")
                for call in sbl["calls"]:
                    qq = call["q"]
                    nc.gpsimd.dma_gather(
                        msg[:, call["mcol"] : call["mcol"] + call["s"] // 128, :],
                        zt2_full.ap()[qq * QS : (qq + 1) * QS, :],
                        idx_t[:, call["ioff16"] : call["ioff16"] + call["s"] // 16],
                        call["s"],
                        call["s"],
                        D_OUT,
                        single_packet=False,
                    )
                pss = {}
                started = {}
                for b in sbl["blocks"]:
                    pss[b] = psumC.tile(
                        [128, D_OUT], f32, tag="agg", name=f"aggC_{b}"
                    )
                    started[b] = False
                mi = 0
                for call in sbl["calls"]:
                    for t, tb in enumerate(call["chunk_blocks"]):
                        col = call["mcol"] + t
                        dl_col = dlc_t[:, sbl["ch0"] + col : sbl["ch0"] + col + 1]
                        for bi_i in tb:
                            b = sbl["blocks"][bi_i]
                            wmask = maskp.tile([128, 128], f16, tag="mask")
                            if mi % 5 < 2:
                                nc.vector.tensor_scalar(
                                    out=wmask[:],
                                    in0=iotaw_t[:, bi_i * 128 : (bi_i + 1) * 128],
                                    scalar1=dl_col, scalar2=None,
                                    op0=mybir.AluOpType.is_equal,
                                )
                            else:
                                # exact one-hot on ScalarE: relu(1-|dl-iota|)
                                adiff = maskp.tile(
                                    [128, 128], f16, tag="adiff"
                                )
                                nc.scalar.activation(
                                    adiff[:],
                                    iotaw_t[:, bi_i * 128 : (bi_i + 1) * 128],
                                    mybir.ActivationFunctionType.Abs,
                                    bias=dl_col, scale=-1.0,
                                )
                                nc.scalar.activation(
                                    wmask[:], adiff[:],
                                    mybir.ActivationFunctionType.Relu,
                                    bias=1.0, scale=-1.0,
                                )
                            mi += 1
                            nc.tensor.matmul(
                                pss[b][:], lhsT=wmask[:],
                                rhs=msg[:, col, :],
                                start=not started[b], stop=False,
                            )
                            started[b] = True
                for b in sbl["blocks"]:
                    self_t = selfp.tile([128, D_OUT], f16, tag="self")
                    nc.sync.dma_start(
                        self_t[:], zt2_c.ap()[b * 128 : (b + 1) * 128, :]
                    )
                    nc.tensor.matmul(
                        pss[b][:], lhsT=ident_t[:], rhs=self_t[:],
                        start=not started[b], stop=True,
                    )
                    t1 = epi.tile([128, D_OUT], f32, tag="t1")
                    nc.vector.tensor_scalar(
                        out=t1[:], in0=pss[b][:],
                        scalar1=dinv_t[:, b : b + 1],
                        scalar2=None, op0=mybir.AluOpType.mult,
                    )
                    t2 = epi.tile([128, D_OUT], f32, tag="t2")
                    nc.vector.tensor_tensor(
                        out=t2[:], in0=t1[:], in1=b2_t[:],
                        op=mybir.AluOpType.add,
                    )
                    nc.sync.dma_start(
                        out[b * 128 : (b + 1) * 128, :], t2[:]
                    )

        consts.release()

    nc.compile()
    return nc


def kernel(x, edge_index, W1, b1, W2, b2):
    from concourse.bass_utils import run_bass_kernel_spmd

    in_maps, layout_info = preprocess(x, edge_index, W1, b1, W2, b2)
    nc = build_nc(layout_info)
    res = run_bass_kernel_spmd(nc, in_maps, core_ids=list(range(N_CORES)))
    outs = [res.results[c]["out"][:NC_NODES] for c in range(N_CORES)]
    return np.concatenate(outs, axis=0).astype(np.float32)


# revision 32
# speedup vs baseline: 1.0104x; 1.0104x over previous
"""Bass/Trainium2 kernel for a 2-layer GCN encoder (PyG GCNConv semantics).

Strategy (graph/data parallel over 8 NeuronCores):
  - Nodes are range-sharded: core c owns dst nodes [c*12500, (c+1)*12500).
  - With the dinv-prescaled features x~ = dinv[:,None]*x and table
    zt2 = dinv[:,None]*(h1 @ W2), each layer is
        h1_i  = relu(dinv_i*((sum_{e->i} x~[src_e] + x~_i) @ W1) + b1)
        out_i = dinv_i*( sum_{e->i} zt2[src_e] + zt2_i ) + b2
    (aggregate-then-transform via linearity for layer 1).
  - Layer 1 messages are HOST-MARSHALLED: x~[src] rows are shipped in
    edge-slot order (halo exchange materialized on the host), so the
    device consumes them with big affine DMAs and scatter-accumulates
    via one-hot matmuls (mask[e,j] = (dst_local[e]==j) built on VectorE
    from a host dst_local stream vs an iota constant, PSUM accumulates).
    Layer-1 slots are packed per dst block (no quarter structure).
  - Layer 2 messages are device-gathered (h1 is device-resident): each
    core computes zt2 for its shard, an AllGather replicates the table
    (fp16), and dma_gather fetches edge-source rows. int16 gather
    indices limit a call to 32767 rows, so the padded 100352-row table
    is split in 4 quarters of 25088 rows. One call per (quarter,
    superblock of SB_C=4 blocks): the four blocks' real edges are
    packed block-major and pads (idx 0, dst_local -1) trail. The Q7
    generation cost is ~8.3ns per slot regardless of validity, so
    packing across 4 blocks minimizes slots. Per chunk ONE wide
    [128, 512] mask op (dst_local value j + 128*bi vs an iota512
    constant) feeds four matmuls, one per block PSUM.
  - All cores run one SPMD NEFF: slot counts are padded to the max over
    cores so the program is identical everywhere.
"""

import sys

import numpy as np

sys.path.insert(0, "/opt/trn_rl_repo")

N_NODES = 100000
N_EDGES = 1600000
D_IN, D_HID, D_OUT = 256, 256, 128
N_CORES = 8
NC_NODES = N_NODES // N_CORES  # 12500 real nodes per core
NP = 12544  # padded nodes per core (98 blocks of 128)
NBLK = NP // 128  # 98
NROWS = N_CORES * NP  # 100352 padded table rows
NQ = 4
QS = NROWS // NQ  # 25088 rows per quarter (< 32767 for int16 idx)
SB_B = 2  # dst blocks per superblock, layer-1 stream batching
SB_C = 4  # dst blocks per superblock, layer-2 gather calls


def _pad_row(n):
    return (n // NC_NODES) * NP + (n % NC_NODES)


def _sbs(sb):
    return [list(range(s, min(s + sb, NBLK))) for s in range(0, NBLK, sb)]


def build_layout_b(counts_b):
    """counts_b: [N_CORES, NBLK] edges per dst block. Layer-1 layout:
    slots packed per block (quarters irrelevant), padded to x128 at the
    max over cores."""
    maxc = counts_b.max(axis=0)
    nch_blk = np.ceil(maxc / 128.0).astype(np.int64)  # chunks per block
    ch_off = np.zeros(NBLK + 1, dtype=np.int64)
    np.cumsum(nch_blk, out=ch_off[1:])
    layout = []
    for blocks in _sbs(SB_B):
        layout.append(
            dict(
                blocks=blocks,
                ch0=int(ch_off[blocks[0]]),
                nch=int(sum(nch_blk[b] for b in blocks)),
                blk_chunks={b: (int(ch_off[b]), int(nch_blk[b])) for b in blocks},
            )
        )
    return dict(
        layout=layout,
        nch=int(ch_off[-1]),
        nslots=int(ch_off[-1]) * 128,
        slot_off=ch_off[:-1] * 128,  # per block
    )


def build_layout_c(cnt_c4):
    """cnt_c4: [N_CORES, NSB, NQ, SB_C] edges per (sb, quarter, block).
    Layer-2 layout: per (sb, q) one gather call; real slots packed
    block-major, pads (idx 0) trail; padded to x128 at the max over
    cores. Each chunk records the union (over cores) of blocks whose
    slot range intersects it, so the device only emits mask+matmul for
    (chunk, block) pairs that can be non-zero on some core."""
    counts_c = cnt_c4.sum(axis=3).transpose(0, 2, 1)  # [cores, NQ, NSB]
    maxc = counts_c.max(axis=0)  # [NQ, NSB]
    s_call = 128 * np.ceil(maxc / 128.0).astype(np.int64)
    layout = []
    gch = 0
    call_id = 0
    sbs = _sbs(SB_C)
    slot_off = np.zeros((NQ, len(sbs)), dtype=np.int64)
    for si, blocks in enumerate(sbs):
        sb_ch0 = gch
        calls = []
        for q in range(NQ):
            s = int(s_call[q][si])
            if s == 0:
                continue
            slot_off[q][si] = gch * 128
            # per-core block boundaries within this call (slot space)
            bounds = np.zeros((N_CORES, len(blocks) + 1), dtype=np.int64)
            np.cumsum(cnt_c4[:, si, q, : len(blocks)], axis=1, out=bounds[:, 1:])
            chunk_blocks = []
            for t in range(s // 128):
                lo, hi = 128 * t, 128 * (t + 1)
                touched = set()
                for c in range(N_CORES):
                    for bi in range(len(blocks)):
                        if bounds[c][bi] < hi and bounds[c][bi + 1] > lo:
                            touched.add(bi)
                chunk_blocks.append(sorted(touched))
            calls.append(
                dict(
                    q=q,
                    ioff16=gch * 8,
                    s=s,
                    mcol=gch - sb_ch0,
                    call_id=call_id,
                    chunk_blocks=chunk_blocks,
                )
            )
            call_id += 1
            gch += s // 128
        layout.append(
            dict(blocks=blocks, ch0=sb_ch0, nch=gch - sb_ch0, calls=calls)
        )
    return dict(
        layout=layout,
        nch=gch,
        nslots=gch * 128,
        ncalls=call_id,
        slot_off=slot_off,
    )


def preprocess(x, edge_index, W1, b1, W2, b2):
    """Host-side sharding/marshalling. Returns (in_maps, (lb, lc))."""
    src = np.asarray(edge_index[0], dtype=np.int64)
    dst = np.asarray(edge_index[1], dtype=np.int64)
    x = np.asarray(x)
    W1 = np.asarray(W1)
    b1 = np.asarray(b1)
    W2 = np.asarray(W2)
    b2 = np.asarray(b2)

    deg = np.bincount(dst, minlength=N_NODES).astype(np.float32) + 1.0
    dinv = (1.0 / np.sqrt(deg)).astype(np.float32)
    xt = (x.astype(np.float32) * dinv[:, None]).astype(np.float16)

    core = dst // NC_NODES
    dstl = dst % NC_NODES
    blk = dstl // 128
    j = (dstl % 128).astype(np.int64)
    prow = _pad_row(src)
    q = prow // QS
    sq = (prow % QS).astype(np.int64)
    sb_of_blk = blk // SB_C
    bi = blk % SB_C
    nsb = (NBLK + SB_C - 1) // SB_C

    # ----- layer-1 (B) layout: group by (core, blk) -----
    key_b = core * NBLK + blk
    order_b = np.argsort(key_b, kind="stable")
    cnt_b = np.bincount(key_b, minlength=N_CORES * NBLK).reshape(N_CORES, NBLK)
    lb = build_layout_b(cnt_b)
    gstart = np.zeros(N_CORES * NBLK + 1, dtype=np.int64)
    np.cumsum(cnt_b.reshape(-1), out=gstart[1:])
    rank_b = np.arange(len(src)) - gstart[key_b[order_b]]
    pos_b = lb["slot_off"][blk[order_b]] + rank_b  # slot in core's B stream

    # ----- layer-2 (C) layout: group by (core, sb, q, bi) -----
    key_c = ((core * nsb + sb_of_blk) * NQ + q) * SB_C + bi
    order_c = np.argsort(key_c, kind="stable")
    ngroups_c = N_CORES * nsb * NQ * SB_C
    cnt_c4 = np.bincount(key_c, minlength=ngroups_c).reshape(
        N_CORES, nsb, NQ, SB_C
    )
    lc = build_layout_c(cnt_c4)
    # rank within (core, sb, q) with blocks in bi order: cumulative offsets
    gstart_c = np.zeros(ngroups_c + 1, dtype=np.int64)
    np.cumsum(cnt_c4.reshape(-1), out=gstart_c[1:])
    key_cs = key_c[order_c]
    grp_base = (key_cs // SB_C) * SB_C  # index of bi=0 group
    off_in_call = gstart_c[key_cs] - gstart_c[grp_base]
    rank_c = np.arange(len(src)) - gstart_c[key_cs]
    pos_c_all = (
        lc["slot_off"][q[order_c], sb_of_blk[order_c]] + off_in_call + rank_c
    )

    in_maps = []
    iota_np = np.tile(np.arange(128, dtype=np.float16)[None, :], (128, 1))
    iotaw_np = np.tile(
        np.arange(128 * SB_C, dtype=np.float16)[None, :], (128, 1)
    )
    ident_np = np.eye(128, dtype=np.float16)
    W1h = W1.astype(np.float16).reshape(D_IN // 128, 128, D_HID).transpose(1, 0, 2).copy()
    W2h = W2.astype(np.float16).reshape(D_HID // 128, 128, D_OUT).transpose(1, 0, 2).copy()
    b1b = np.tile(b1.astype(np.float32)[None, :], (128, 1))
    b2b = np.tile(b2.astype(np.float32)[None, :], (128, 1))

    core_bs = core[order_b]
    core_cs = core[order_c]

    for c in range(N_CORES):
        # --- B stream: marshalled x~[src] rows + dst_local values ---
        m_b = core_bs == c
        posb = pos_b[m_b]
        nslB, nchB = lb["nslots"], lb["nch"]
        dlb = np.full(nslB, -1.0, dtype=np.float32)
        dlb[posb] = j[order_b][m_b].astype(np.float32)
        dlb_w = dlb.reshape(-1, 128).T.copy()
        xs = np.zeros((nslB, D_IN), dtype=np.float16)
        xs[posb] = xt[src[order_b][m_b]]
        xs = np.ascontiguousarray(xs.reshape(nchB, 128, D_IN).transpose(1, 0, 2))

        # --- C stream: gather idx + dual-block dst_local + counts ---
        m_c = core_cs == c
        posc = pos_c_all[m_c]
        nslC, nchC = lc["nslots"], lc["nch"]
        idxc = np.zeros(nslC, dtype=np.int16)
        idxc[posc] = sq[order_c][m_c].astype(np.int16)
        dlc = np.full(nslC, -1.0, dtype=np.float32)
        dlc[posc] = (j[order_c][m_c] + 128 * bi[order_c][m_c]).astype(
            np.float32
        )
        dlc_w = dlc.reshape(-1, 128).T.copy()
        idxc_w = np.tile(idxc.reshape(-1, 16).T, (8, 1)).copy()

        # --- own-shard x~ rows for the L1 self term ---
        xself = np.zeros((NP, D_IN), dtype=np.float16)
        xself[:NC_NODES] = xt[c * NC_NODES : (c + 1) * NC_NODES]
        xself = np.ascontiguousarray(
            xself.reshape(NBLK, 128, D_IN).transpose(1, 0, 2)
        )

        dinv_c = np.zeros((128, NBLK), dtype=np.float32)
        dv = np.zeros(NP, dtype=np.float32)
        dv[:NC_NODES] = dinv[c * NC_NODES : (c + 1) * NC_NODES]
        dinv_c[:, :] = dv.reshape(NBLK, 128).T

        in_maps.append(
            dict(
                xs=xs,
                xself=xself,
                W1h=W1h,
                W2h=W2h,
                b1b=b1b,
                b2b=b2b,
                iota=iota_np,
                iotaw=iotaw_np,
                ident=ident_np,
                dinv=dinv_c,
                eidx=idxc_w,
                edlB=dlb_w,
                edlC=dlc_w,
            )
        )
    return in_maps, (lb, lc)


def build_nc(layout_info):
    import concourse.tile as tile
    from concourse import bacc, mybir

    lb, lc = layout_info
    nchB = lb["nch"]
    nchC, nslotsC = lc["nch"], lc["nslots"]
    f16 = mybir.dt.float16
    f32 = mybir.dt.float32
    i16 = mybir.dt.int16

    nc = bacc.Bacc(
        "TRN2", target_bir_lowering=False, debug=False, num_devices=N_CORES
    )
    xs = nc.dram_tensor("xs", [128, nchB, D_IN], f16, kind="ExternalInput").ap()
    xself = nc.dram_tensor("xself", [128, NBLK, D_IN], f16, kind="ExternalInput").ap()
    W1h = nc.dram_tensor("W1h", [128, D_IN // 128, D_HID], f16, kind="ExternalInput").ap()
    W2h = nc.dram_tensor("W2h", [128, D_HID // 128, D_OUT], f16, kind="ExternalInput").ap()
    b1b = nc.dram_tensor("b1b", [128, D_HID], f32, kind="ExternalInput").ap()
    b2b = nc.dram_tensor("b2b", [128, D_OUT], f32, kind="ExternalInput").ap()
    iota = nc.dram_tensor("iota", [128, 128], f16, kind="ExternalInput").ap()
    iotaw = nc.dram_tensor("iotaw", [128, 128 * SB_C], f16, kind="ExternalInput").ap()
    ident = nc.dram_tensor("ident", [128, 128], f16, kind="ExternalInput").ap()
    dinv = nc.dram_tensor("dinv", [128, NBLK], f32, kind="ExternalInput").ap()
    eidx = nc.dram_tensor("eidx", [128, nslotsC // 16], i16, kind="ExternalInput").ap()
    edlB = nc.dram_tensor("edlB", [128, nchB], f32, kind="ExternalInput").ap()
    edlC = nc.dram_tensor("edlC", [128, nchC], f32, kind="ExternalInput").ap()
    out = nc.dram_tensor("out", [NP, D_OUT], f32, kind="ExternalOutput").ap()

    zt2_c = nc.dram_tensor("zt2_c", [NP, D_OUT], f16)
    zt2_full = nc.dram_tensor("zt2_full", [NROWS, D_OUT], f16, addr_space="Shared")

    with tile.TileContext(nc) as tc:
        consts = tc.alloc_tile_pool(name="consts", bufs=1)
        w1_t = consts.tile([128, D_IN // 128, D_HID], f16)
        nc.sync.dma_start(w1_t[:], W1h[:, :, :])
        w2_t = consts.tile([128, D_HID // 128, D_OUT], f16)
        nc.sync.dma_start(w2_t[:], W2h[:, :, :])
        b1_t = consts.tile([128, D_HID], f32)
        nc.sync.dma_start(b1_t[:], b1b[:, :])
        b2_t = consts.tile([128, D_OUT], f32)
        nc.sync.dma_start(b2_t[:], b2b[:, :])
        iota_t = consts.tile([128, 128], f16)
        nc.sync.dma_start(iota_t[:], iota[:, :])
        iotaw_t = consts.tile([128, 128 * SB_C], f16)
        nc.sync.dma_start(iotaw_t[:], iotaw[:, :])
        ident_t = consts.tile([128, 128], f16)
        nc.sync.dma_start(ident_t[:], ident[:, :])
        dinv_t = consts.tile([128, NBLK], f32)
        nc.sync.dma_start(dinv_t[:], dinv[:, :])
        dlb_t = consts.tile([128, nchB], f32)
        nc.sync.dma_start(dlb_t[:], edlB[:, :])

        def make_mask(maskp, dl_t, gc, base_t):
            mask = maskp.tile([128, 128], f16, tag="mask")
            nc.vector.tensor_scalar(
                out=mask[:], in0=base_t[:], scalar1=dl_t[:, gc : gc + 1],
                scalar2=None, op0=mybir.AluOpType.is_equal,
            )
            return mask

        # ------------- Phase B: L1 aggregate-then-transform + zt2 -------------
        with tc.tile_pool(name="msgB", bufs=4) as msgp, \
             tc.tile_pool(name="maskB", bufs=10) as maskp, \
             tc.tile_pool(name="selfB", bufs=4) as selfp, \
             tc.tile_pool(name="psumX", bufs=2, space="PSUM") as psumX, \
             tc.tile_pool(name="psumT", bufs=2, space="PSUM") as psumT, \
             tc.tile_pool(name="psumZ", bufs=2, space="PSUM") as psumZ, \
             tc.tile_pool(name="epiB", bufs=6) as epi:
            for sbl in lb["layout"]:
                msg = msgp.tile([128, sbl["nch"], D_IN], f16, tag="msg")
                nc.sync.dma_start(
                    msg[:], xs[:, sbl["ch0"] : sbl["ch0"] + sbl["nch"], :]
                )
                for b in sbl["blocks"]:
                    ch0, nch_b = sbl["blk_chunks"][b]
                    lc0 = ch0 - sbl["ch0"]
                    self_t = selfp.tile([128, D_IN], f16, tag="self")
                    nc.sync.dma_start(self_t[:], xself[:, b, :])
                    psx = psumX.tile([128, D_IN], f32, tag="aggx")
                    for t in range(nch_b):
                        if t % 4 != 3:
                            mask = make_mask(maskp, dlb_t, ch0 + t, iota_t)
                        else:
                            dl_col = dlb_t[:, ch0 + t : ch0 + t + 1]
                            adiff = maskp.tile([128, 128], f16, tag="adiffB")
                            nc.scalar.activation(
                                adiff[:], iota_t[:],
                                mybir.ActivationFunctionType.Abs,
                                bias=dl_col, scale=-1.0,
                            )
                            mask = maskp.tile([128, 128], f16, tag="mask")
                            nc.scalar.activation(
                                mask[:], adiff[:],
                                mybir.ActivationFunctionType.Relu,
                                bias=1.0, scale=-1.0,
                            )
                        nc.tensor.matmul(
                            psx[:], lhsT=mask[:], rhs=msg[:, lc0 + t, :],
                            start=(t == 0), stop=False,
                        )
                    nc.tensor.matmul(
                        psx[:], lhsT=ident_t[:], rhs=self_t[:],
                        start=(nch_b == 0), stop=True,
                    )
                    # aggx (psum f32) -> fp16 sbuf -> transpose -> @W1
                    aggx = epi.tile([128, D_IN], f16, tag="aggx16")
                    nc.scalar.activation(
                        aggx[:], psx[:], mybir.ActivationFunctionType.Copy
                    )
                    aggxT = epi.tile([128, D_IN // 128, 128], f16, tag="aggxT")
                    for k in range(D_IN // 128):
                        pst = psumT.tile([128, 128], f16, tag="pst")
                        nc.tensor.transpose(
                            pst[:], aggx[:, k * 128 : (k + 1) * 128], ident_t[:]
                        )
                        nc.scalar.activation(
                            aggxT[:, k, :], pst[:],
                            mybir.ActivationFunctionType.Copy,
                        )
                    psz = psumZ.tile([128, D_HID], f32, tag="psz")
                    for k in range(D_IN // 128):
                        nc.tensor.matmul(
                            psz[:], lhsT=aggxT[:, k, :], rhs=w1_t[:, k, :],
                            start=(k == 0), stop=(k == D_IN // 128 - 1),
                        )
                    # h1 = relu(dinv * psz + b1)
                    t1 = epi.tile([128, D_HID], f32, tag="t1")
                    nc.vector.tensor_scalar(
                        out=t1[:], in0=psz[:], scalar1=dinv_t[:, b : b + 1],
                        scalar2=None, op0=mybir.AluOpType.mult,
                    )
                    nc.vector.tensor_tensor(
                        out=t1[:], in0=t1[:], in1=b1_t[:], op=mybir.AluOpType.add
                    )
                    h1 = epi.tile([128, D_HID], f16, tag="h1")
                    nc.scalar.activation(
                        h1[:], t1[:], mybir.ActivationFunctionType.Relu
                    )
                    # zt2 = dinv * (h1 @ W2)
                    h1T = epi.tile([128, D_HID // 128, 128], f16, tag="h1T")
                    for k in range(D_HID // 128):
                        pst = psumT.tile([128, 128], f16, tag="pst")
                        nc.tensor.transpose(
                            pst[:], h1[:, k * 128 : (k + 1) * 128], ident_t[:]
                        )
                        nc.scalar.activation(
                            h1T[:, k, :], pst[:],
                            mybir.ActivationFunctionType.Copy,
                        )
                    ps2 = psumZ.tile([128, D_OUT], f32, tag="ps2")
                    for k in range(D_HID // 128):
                        nc.tensor.matmul(
                            ps2[:], lhsT=h1T[:, k, :], rhs=w2_t[:, k, :],
                            start=(k == 0), stop=(k == D_HID // 128 - 1),
                        )
                    zt2 = epi.tile([128, D_OUT], f16, tag="zt2")
                    nc.vector.tensor_scalar(
                        out=zt2[:], in0=ps2[:], scalar1=dinv_t[:, b : b + 1],
                        scalar2=None, op0=mybir.AluOpType.mult,
                    )
                    nc.sync.dma_start(
                        zt2_c.ap()[b * 128 : (b + 1) * 128, :], zt2[:]
                    )

        tc.strict_bb_all_engine_barrier()
        with tc.tile_critical():
            with nc.semaphore("cc2") as cc2:
                nc.gpsimd.collective_compute(
                    "AllGather",
                    mybir.AluOpType.bypass,
                    replica_groups=[list(range(N_CORES))],
                    ins=[zt2_c.ap().opt()],
                    outs=[zt2_full.ap().opt()],
                ).then_inc(cc2)
                nc.gpsimd.wait_ge(cc2, 1)
        tc.strict_bb_all_engine_barrier()

        # ---------------- Phase C: L2 aggregation -> out ----------------
        idx_t = consts.tile([128, nslotsC // 16], i16)
        nc.sync.dma_start(idx_t[:], eidx[:, :])
        dlc_t = consts.tile([128, nchC], f32)
        nc.sync.dma_start(dlc_t[:], edlC[:, :])

        with tc.tile_pool(name="msgC", bufs=4) as msgp, \
             tc.tile_pool(name="maskC", bufs=10) as maskp, \
             tc.tile_pool(name="selfC", bufs=4) as selfp, \
             tc.tile_pool(name="psumC", bufs=2 * SB_C, space="PSUM") as psumC, \
             tc.tile_pool(name="epiC", bufs=4) as epi:
            for sbi, sbl in enumerate(lc["layout"]):
                msg = msgp.tile([128, sbl["nch"], D_OUT], f16, tag="msg")
                for call in sbl["calls"]:
                    qq = call["q"]
                    nc.gpsimd.dma_gather(
                        msg[:, call["mcol"] : call["mcol"] + call["s"] // 128, :],
                        zt2_full.ap()[qq * QS : (qq + 1) * QS, :],
                        idx_t[:, call["ioff16"] : call["ioff16"] + call["s"] // 16],
                        call["s"],
                        call["s"],
                        D_OUT,
                        single_packet=False,
                    )
                pss = {}
                started = {}
                for b in sbl["blocks"]:
                    pss[b] = psumC.tile(
                        [128, D_OUT], f32, tag="agg", name=f"aggC_{b}"
                    )
                    started[b] = False
                mi = 0
                for call in sbl["calls"]:
                    for t, tb in enumerate(call["chunk_blocks"]):
                        col = call["mcol"] + t
                        dl_col = dlc_t[:, sbl["ch0"] + col : sbl["ch0"] + col + 1]
                        for bi_i in tb:
                            b = sbl["blocks"][bi_i]
                            wmask = maskp.tile([128, 128], f16, tag="mask")
                            if mi % 5 < 2:
                                nc.vector.tensor_scalar(
                                    out=wmask[:],
                                    in0=iotaw_t[:, bi_i * 128 : (bi_i + 1) * 128],
                                    scalar1=dl_col, scalar2=None,
                                    op0=mybir.AluOpType.is_equal,
                                )
                            else:
                                # exact one-hot on ScalarE: relu(1-|dl-iota|)
                                adiff = maskp.tile(
                                    [128, 128], f16, tag="adiff"
                                )
                                nc.scalar.activation(
                                    adiff[:],
                                    iotaw_t[:, bi_i * 128 : (bi_i + 1) * 128],
                                    mybir.ActivationFunctionType.Abs,
                                    bias=dl_col, scale=-1.0,
                                )
                                nc.scalar.activation(
                                    wmask[:], adiff[:],
                                    mybir.ActivationFunctionType.Relu,
                                    bias=1.0, scale=-1.0,
                                )
                            mi += 1
                            nc.tensor.matmul(
                                pss[b][:], lhsT=wmask[:],
                                rhs=msg[:, col, :],
                                start=not started[b], stop=False,
                            )
                            started[b] = True
                for b in sbl["blocks"]:
                    self_t = selfp.tile([128, D_OUT], f16, tag="self")
                    nc.sync.dma_start(
                        self_t[:], zt2_c.ap()[b * 128 : (b + 1) * 128, :]
                    )
                    nc.tensor.matmul(
                        pss[b][:], lhsT=ident_t[:], rhs=self_t[:],
                        start=not started[b], stop=True,
                    )
                    t1 = epi.tile([128, D_OUT], f32, tag="t1")
                    nc.vector.tensor_scalar(
                        out=t1[:], in0=pss[b][:],
                        scalar1=dinv_t[:, b : b + 1],
                        scalar2=None, op0=mybir.AluOpType.mult,
                    )
                    t2 = epi.tile([128, D_OUT], f32, tag="t2")
                    nc.vector.tensor_tensor(
                        out=t2[:], in0=t1[:], in1=b2_t[:],
                        op=mybir.AluOpType.add,
                    )
                    nc.sync.dma_start(
                        out[b * 128 : (b + 1) * 128, :], t2[:]
                    )

        consts.release()

    nc.compile()
    return nc


def kernel(x, edge_index, W1, b1, W2, b2):
    from concourse.bass_utils import run_bass_kernel_spmd

    in_maps, layout_info = preprocess(x, edge_index, W1, b1, W2, b2)
    nc = build_nc(layout_info)
    res = run_bass_kernel_spmd(nc, in_maps, core_ids=list(range(N_CORES)))
    outs = [res.results[c]["out"][:NC_NODES] for c in range(N_CORES)]
    return np.concatenate(outs, axis=0).astype(np.float32)
